# revision 7
# baseline (speedup 1.0000x reference)
"""Trainium2 Bass kernel for nn_Mem_Conv2d: 3x3 same-pad conv, NCHW
x[16,32,256,256] (*) crossbar-quantized weight[32,32,3,3] + bias[32].

Strategy
--------
- Data-parallel over batch: 16 images -> 8 cores x 2 images.
- Per core, 4 SBUF partition groups g = (image n, half h): 32 channels each.
- Conv as 9 shifted matmuls (taps) accumulating in PSUM; shifts are pure
  free-dim AP offsets into a zero-padded row strip (no data movement).
- 16-way TensorE array packing (32x32 tiles): row group = g (4 strips),
  col group = 4 output-row windows; 9 taps accumulate serially per position.
- Precision: weights are the crossbar-quantized values = s * k with k an
  integer in [-7,7] (exact in bf16); x is split into bf16 high+low parts
  (x ~= xh + xl, residual ~2^-17). Two bf16 matmul terms give ~1e-5 rel err
  at 4x the fp32 matmul rate. The scale s is folded into the PSUM eviction
  (ACT Identity: out = psum * s + bias).
- Output: ACT evicts PSUM->SBUF with scale+bias, out-DMA on the ACT HWDGE
  ring scatters [4 rows x 32 och x 256] to HBM.

Modes (env BASS_CONV_MODE): "split2" (default), "bf16" (single term),
"fp32" (exact fp32 matmuls, 4 cyc/row).
"""

import os
import numpy as np

import concourse.bacc as bacc
import concourse.mybir as mybir
from concourse.tile import TileContext
from concourse.bass_utils import run_bass_kernel_spmd

import ml_dtypes

# problem geometry (hardcoded per harness contract)
N_IMG, C, H, W = 16, 32, 256, 256
N_CORES = 8
IMG_PER_CORE = 2
W_PAD = W + 2          # 258
ROWS_SLOT = 34         # 32 output rows + 2 halo
SLOT_FREE = ROWS_SLOT * W_PAD  # 8772
PHASES = 4             # 4 x 32 rows = 128 rows per half
SUPERS = 8             # super-iters per phase (4 output rows each)
NWIN = 4               # windows (col groups) per super-iter
NGRP = 4               # partition groups (row groups)
RES_SLOTS = 8

QMAX = 7.0


def _mode():
    return os.environ.get("BASS_CONV_MODE", "split2")


def build_nc(mode):
    f32 = mybir.dt.float32
    bf16 = mybir.dt.bfloat16
    mm_dt = f32 if mode == "fp32" else bf16
    nterm = 2 if mode == "split2" else 1

    nc = bacc.Bacc("TRN2", target_bir_lowering=False)
    x_d = nc.dram_tensor("x", [IMG_PER_CORE, C, H, W], f32, kind="ExternalInput")
    w_d = nc.dram_tensor("w", [128, 9 * 32], mm_dt, kind="ExternalInput")
    b_d = nc.dram_tensor("b", [128, 1], f32, kind="ExternalInput")
    s_d = nc.dram_tensor("s", [128, 1], f32, kind="ExternalInput")
    o_d = nc.dram_tensor("o", [IMG_PER_CORE, C, H, W], f32, kind="ExternalOutput")

    NF32 = 2           # fp32 strip slots
    NB = 3             # bf16 strip slots

    with TileContext(nc) as tc:
        with (
            tc.tile_pool(name="sb", bufs=1) as sb,
            tc.tile_pool(name="ps", bufs=2, space="PSUM") as ps,
        ):
            xf = sb.tile([128, NF32 * SLOT_FREE + 8], f32)
            wt = sb.tile([128, 9 * 32], mm_dt)
            bt = sb.tile([128, 1], f32)
            st = sb.tile([128, 1], f32)
            res = sb.tile([128, RES_SLOTS * W_PAD], f32)
            if mode != "fp32":
                xh = sb.tile([128, NB * SLOT_FREE + 8], bf16)
                if mode == "split2":
                    xl = sb.tile([128, NB * SLOT_FREE + 8], bf16)

            nc.sync.dma_start(out=wt[:, :], in_=w_d[:, :])
            nc.sync.dma_start(out=bt[:, :], in_=b_d[:, :])
            nc.sync.dma_start(out=st[:, :], in_=s_d[:, :])

            # zero the horizontal pad columns (cols 0 and 257 of each row) of
            # every fp32 slot once; DMAs never write them so they stay zero.
            padv = xf[:, 0 : NF32 * SLOT_FREE].rearrange(
                "p (s r q) -> p s r q", s=NF32, r=ROWS_SLOT
            )
            nc.gpsimd.memset(padv[:, :, :, 0:258:257], 0.0)
            # slack tail (read-overrun area of last slot's last window)
            nc.gpsimd.memset(xf[:, NF32 * SLOT_FREE :], 0.0)
            if mode != "fp32":
                # bf16 slack: never written by splits, read into junk cols only
                nc.gpsimd.memset(xh[:, NB * SLOT_FREE :], 0.0)
                if mode == "split2":
                    nc.gpsimd.memset(xl[:, NB * SLOT_FREE :], 0.0)
            # phase 0, h=0 groups: top image row halo must be zero (slot 0 row 0)
            v0 = xf[:, 0:SLOT_FREE].rearrange(
                "(n h c) (r q) -> n h c r q", n=2, h=2, r=ROWS_SLOT
            )


            def slot_f32(p):
                return (p % NF32) * SLOT_FREE

            def slot_b(p):
                return (p % NB) * SLOT_FREE

            sg = 0  # global super-iter counter
            reps = int(os.environ.get("BASS_CONV_REPS", "1"))
            for p in [pp for _ in range(reps) for pp in range(PHASES)]:
                off = slot_f32(p)
                vslot = xf[:, off : off + SLOT_FREE].rearrange(
                    "p (r q) -> p r q", r=ROWS_SLOT
                )
                vslot4 = xf[:, off : off + SLOT_FREE].rearrange(
                    "(n h c) (r q) -> n h c r q", n=2, h=2, r=ROWS_SLOT
                )
                r0 = 32 * p - 1
                if p == 0:
                    for n in range(2):
                        nc.gpsimd.memset(vslot4[n, 0, :, 0, :], 0.0)
                if p == PHASES - 1:
                    # h=1 groups need bottom halo zero at slot row 33
                    for n in range(2):
                        nc.gpsimd.memset(vslot4[n, 1, :, 33, :], 0.0)
                for n in range(2):
                    for h in range(2):
                        a = 128 * h + r0  # first absolute input row of the slot
                        lo = max(a, 0)
                        hi = min(a + ROWS_SLOT, H)
                        nc.sync.dma_start(
                            out=vslot4[n, h, :, lo - a : hi - a, 1:257],
                            in_=x_d[n, :, lo:hi, :],
                        )

                # split x into bf16 high/low parts (whole slot, all 128 parts)
                if mode != "fp32":
                    boff = slot_b(p)
                    nc.scalar.activation(
                        xh[:, boff : boff + SLOT_FREE],
                        xf[:, off : off + SLOT_FREE],
                        mybir.ActivationFunctionType.Copy,
                    )
                    if mode == "split2":
                        nc.vector.tensor_sub(
                            xl[:, boff : boff + SLOT_FREE],
                            xf[:, off : off + SLOT_FREE],
                            xh[:, boff : boff + SLOT_FREE],
                        )
                    terms = [xh, xl][:nterm]
                    toffs = [boff] * nterm
                else:
                    terms = [xf]
                    toffs = [off]

                for s in range(SUPERS):
                    pts = []
                    for g in range(NGRP):
                        pt = ps.tile([128, W_PAD], f32, name=f"pt{g}", tag=f"pt{g}")
                        pts.append(pt)
                    nmm = nterm * 9
                    mi = 0
                    for ti in range(nterm):
                        src = terms[ti]
                        toff = toffs[ti]
                        for t in range(9):
                            ky, kx = divmod(t, 3)
                            for g in range(NGRP):
                                for c in range(NWIN):
                                    j = 4 * s + c
                                    base = toff + (j + ky) * W_PAD + kx
                                    nc.tensor.matmul(
                                        pts[g][32 * c : 32 * c + 32, :],
                                        wt[32 * g : 32 * g + 32, 32 * t : 32 * t + 32],
                                        src[32 * g : 32 * g + 32, base : base + W_PAD],
                                        start=(mi == 0),
                                        stop=(mi == nmm - 1),
                                        tile_position=(32 * g, 32 * c),
                                    )
                            mi += 1

                    for g in range(NGRP):
                        n, h = g // 2, g % 2
                        slot = (sg * NGRP + g) % RES_SLOTS
                        rs = res[:, slot * W_PAD : (slot + 1) * W_PAD]
                        nc.scalar.activation(
                            rs,
                            pts[g][:, :],
                            mybir.ActivationFunctionType.Identity,
                            bias=bt[:, :],
                            scale=st[:, :],
                        )
                        y0 = 128 * h + 32 * p + 4 * s
                        nc.scalar.dma_start(
                            out=o_d[n, :, y0 : y0 + 4, :].rearrange(
                                "o y w -> y o w"
                            ),
                            in_=rs[:, 0:256],
                        )
                    sg += 1
    nc.finalize()
    return nc


_NC_CACHE = {}


def _get_nc(mode):
    key = (mode, os.environ.get("BASS_CONV_REPS", "1"))
    if key not in _NC_CACHE:
        _NC_CACHE[key] = build_nc(mode)
    return _NC_CACHE[key]


def _host_prep(weight, bias, mode):
    W32 = np.asarray(weight, dtype=np.float32)
    wmax = np.float32(np.max(np.abs(W32))) + np.float32(1e-12)
    k = np.round((W32 / wmax) * np.float32(QMAX))  # integral, in [-7, 7]

    if mode == "fp32":
        qW = (k / np.float32(QMAX)) * wmax  # bit-matches reference crossbar_map
        wvals = qW.astype(np.float32)
        scale = np.float32(1.0)
        np_dt = np.float32
    else:
        wvals = k  # exact small integers
        scale = np.float32(np.float64(wmax) / QMAX)
        np_dt = ml_dtypes.bfloat16

    # lhsT layout: [i, t*32 + o], t = 3*ky + kx
    lhsT = wvals.transpose(1, 2, 3, 0).reshape(C, 9 * C)  # [i,(ky,kx,o)]
    w_rep = np.tile(lhsT, (4, 1)).astype(np_dt)
    b_rep = np.tile(np.asarray(bias, dtype=np.float32)[:, None], (4, 1))
    s_rep = np.full((128, 1), scale, dtype=np.float32)
    return w_rep, b_rep, s_rep


def kernel(x, weight, bias):
    mode = _mode()
    x = np.asarray(x, dtype=np.float32)
    w_rep, b_rep, s_rep = _host_prep(weight, bias, mode)
    nc = _get_nc(mode)

    in_maps = []
    for cid in range(N_CORES):
        in_maps.append(
            {
                "x": np.ascontiguousarray(x[2 * cid : 2 * cid + 2]),
                "w": w_rep,
                "b": b_rep,
                "s": s_rep,
            }
        )
    r = run_bass_kernel_spmd(nc, in_maps, list(range(N_CORES)))
    out = np.empty((N_IMG, C, H, W), dtype=np.float32)
    for cid in range(N_CORES):
        out[2 * cid : 2 * cid + 2] = r.results[cid]["o"]
    return out


# revision 10
# speedup vs baseline: 78.1482x; 78.1482x over previous
"""Trainium2 Bass kernel for nn_Mem_Conv2d: 3x3 same-pad conv, NCHW
x[16,32,256,256] (*) crossbar-quantized weight[32,32,3,3] + bias[32].

Strategy
--------
- Data-parallel over batch: 16 images -> 8 cores x 2 images.
- Per core, 4 SBUF partition groups g = (image n, half h): 32 channels each.
- Conv as 9 shifted matmuls (taps) accumulating in PSUM; shifts are pure
  free-dim AP offsets into a zero-padded row strip (no data movement).
- 16-way TensorE array packing (32x32 tiles): row group = g (4 strips),
  col group = 4 output-row windows; 9 taps accumulate serially per position.
- Precision: weights are the crossbar-quantized values = s * k with k an
  integer in [-7,7] (exact in bf16); x is split into bf16 high+low parts
  (x ~= xh + xl, residual ~2^-17). Two bf16 matmul terms give ~1e-5 rel err
  at 4x the fp32 matmul rate. The scale s is folded into the PSUM eviction
  (ACT Identity: out = psum * s + bias).
- Output: ACT evicts PSUM->SBUF with scale+bias, out-DMA on the ACT HWDGE
  ring scatters [4 rows x 32 och x 256] to HBM.

Modes (env BASS_CONV_MODE): "fp32" (default; exact fp32 matmuls, weights
bit-match the reference crossbar_map), "split2" (bf16 high+low split,
~1.4e-5 abs err, similar speed), "bf16" (single term).
"""

import os
import numpy as np

import concourse.bacc as bacc
import concourse.mybir as mybir
from concourse.tile import TileContext
from concourse.bass_utils import run_bass_kernel_spmd

import ml_dtypes

# walrus disables its LDWEIGHTS optimization by default; our kernel issues
# 9216 ldweights+matmul pairs, so LDW hoisting/background-buffering matters.
# Rewrite the flag at compile time (opt-in via BASS_LDW_OPT=1).
import concourse.bass_utils as _bu

if not getattr(_bu, "_ldw_patch", False):
    _orig_run_command = _bu.run_command

    def _patched_run_command(cmd, *a, **kw):
        if os.environ.get("BASS_LDW_OPT", "0") == "1" and isinstance(cmd, list):
            cmd = [
                "--enable-ldw-opt=true" if c == "--enable-ldw-opt=false" else c
                for c in cmd
            ]
        return _orig_run_command(cmd, *a, **kw)

    _bu.run_command = _patched_run_command
    _bu._ldw_patch = True

# problem geometry (hardcoded per harness contract)
N_IMG, C, H, W = 16, 32, 256, 256
N_CORES = 8
IMG_PER_CORE = 2
W_PAD = W + 2          # 258
ROWS_SLOT = 34         # 32 output rows + 2 halo
SLOT_FREE = ROWS_SLOT * W_PAD  # 8772
PHASES = 4             # 4 x 32 rows = 128 rows per half
SUPERS = 8             # super-iters per phase (4 output rows each)
NWIN = 4               # windows (col groups) per super-iter
NGRP = 4               # partition groups (row groups)
RES_SLOTS = 8

QMAX = 7.0


def _mode():
    return os.environ.get("BASS_CONV_MODE", "fp32")


def build_nc(mode):
    f32 = mybir.dt.float32
    bf16 = mybir.dt.bfloat16
    mm_dt = f32 if mode == "fp32" else bf16
    nterm = 2 if mode == "split2" else 1

    nc = bacc.Bacc("TRN2", target_bir_lowering=False)
    x_d = nc.dram_tensor("x", [IMG_PER_CORE, C, H, W], f32, kind="ExternalInput")
    w_d = nc.dram_tensor("w", [128, 9 * 32], mm_dt, kind="ExternalInput")
    b_d = nc.dram_tensor("b", [128, 1], f32, kind="ExternalInput")
    s_d = nc.dram_tensor("s", [128, 1], f32, kind="ExternalInput")
    o_d = nc.dram_tensor("o", [IMG_PER_CORE, C, H, W], f32, kind="ExternalOutput")

    NF32 = 2           # fp32 strip slots
    NB = 3             # bf16 strip slots

    with TileContext(nc) as tc:
        with (
            tc.tile_pool(name="sb", bufs=1) as sb,
            tc.tile_pool(name="ps", bufs=2, space="PSUM") as ps,
        ):
            xf = sb.tile([128, NF32 * SLOT_FREE + 8], f32)
            wt = sb.tile([128, 9 * 32], mm_dt)
            bt = sb.tile([128, 1], f32)
            st = sb.tile([128, 1], f32)
            res = sb.tile([128, RES_SLOTS * W_PAD], f32)
            if mode != "fp32":
                xh = sb.tile([128, NB * SLOT_FREE + 8], bf16)
                if mode == "split2":
                    xl = sb.tile([128, NB * SLOT_FREE + 8], bf16)

            nc.sync.dma_start(out=wt[:, :], in_=w_d[:, :])
            nc.sync.dma_start(out=bt[:, :], in_=b_d[:, :])
            nc.sync.dma_start(out=st[:, :], in_=s_d[:, :])

            # zero the horizontal pad columns (cols 0 and 257 of each row) of
            # every fp32 slot once; DMAs never write them so they stay zero.
            padv = xf[:, 0 : NF32 * SLOT_FREE].rearrange(
                "p (s r q) -> p s r q", s=NF32, r=ROWS_SLOT
            )
            nc.gpsimd.memset(padv[:, :, :, 0:258:257], 0.0)
            # slack tail (read-overrun area of last slot's last window)
            nc.gpsimd.memset(xf[:, NF32 * SLOT_FREE :], 0.0)
            if mode != "fp32":
                # bf16 slack: never written by splits, read into junk cols only
                nc.gpsimd.memset(xh[:, NB * SLOT_FREE :], 0.0)
                if mode == "split2":
                    nc.gpsimd.memset(xl[:, NB * SLOT_FREE :], 0.0)
            # phase 0, h=0 groups: top image row halo must be zero (slot 0 row 0)
            v0 = xf[:, 0:SLOT_FREE].rearrange(
                "(n h c) (r q) -> n h c r q", n=2, h=2, r=ROWS_SLOT
            )


            def slot_f32(p):
                return (p % NF32) * SLOT_FREE

            def slot_b(p):
                return (p % NB) * SLOT_FREE

            sg = 0  # global super-iter counter
            reps = int(os.environ.get("BASS_CONV_REPS", "1"))
            for p in [pp for _ in range(reps) for pp in range(PHASES)]:
                off = slot_f32(p)
                vslot = xf[:, off : off + SLOT_FREE].rearrange(
                    "p (r q) -> p r q", r=ROWS_SLOT
                )
                vslot4 = xf[:, off : off + SLOT_FREE].rearrange(
                    "(n h c) (r q) -> n h c r q", n=2, h=2, r=ROWS_SLOT
                )
                r0 = 32 * p - 1
                if p == 0:
                    for n in range(2):
                        nc.gpsimd.memset(vslot4[n, 0, :, 0, :], 0.0)
                if p == PHASES - 1:
                    # h=1 groups need bottom halo zero at slot row 33
                    for n in range(2):
                        nc.gpsimd.memset(vslot4[n, 1, :, 33, :], 0.0)
                for n in range(2):
                    for h in range(2):
                        a = 128 * h + r0  # first absolute input row of the slot
                        lo = max(a, 0)
                        hi = min(a + ROWS_SLOT, H)
                        nc.sync.dma_start(
                            out=vslot4[n, h, :, lo - a : hi - a, 1:257],
                            in_=x_d[n, :, lo:hi, :],
                        )

                # split x into bf16 high/low parts (whole slot, all 128 parts)
                if mode != "fp32":
                    boff = slot_b(p)
                    nc.scalar.activation(
                        xh[:, boff : boff + SLOT_FREE],
                        xf[:, off : off + SLOT_FREE],
                        mybir.ActivationFunctionType.Copy,
                    )
                    if mode == "split2":
                        nc.vector.tensor_sub(
                            xl[:, boff : boff + SLOT_FREE],
                            xf[:, off : off + SLOT_FREE],
                            xh[:, boff : boff + SLOT_FREE],
                        )
                    terms = [xh, xl][:nterm]
                    toffs = [boff] * nterm
                else:
                    terms = [xf]
                    toffs = [off]

                for s in range(SUPERS):
                    pts = []
                    for g in range(NGRP):
                        pt = ps.tile([128, W_PAD], f32, name=f"pt{g}", tag=f"pt{g}")
                        pts.append(pt)
                    nmm = nterm * 9
                    mi = 0
                    order = os.environ.get("BASS_CONV_ORDER", "gc")
                    for ti in range(nterm):
                        src = terms[ti]
                        toff = toffs[ti]
                        for t in range(9):
                            ky, kx = divmod(t, 3)
                            gc = (
                                [(g, c) for g in range(NGRP) for c in range(NWIN)]
                                if order == "gc"
                                else [(g, c) for c in range(NWIN) for g in range(NGRP)]
                            )
                            for g, c in gc:
                                j = 4 * s + c
                                base = toff + (j + ky) * W_PAD + kx
                                nc.tensor.matmul(
                                    pts[g][32 * c : 32 * c + 32, :],
                                    wt[32 * g : 32 * g + 32, 32 * t : 32 * t + 32],
                                    src[32 * g : 32 * g + 32, base : base + W_PAD],
                                    start=(mi == 0),
                                    stop=(mi == nmm - 1),
                                    tile_position=(32 * g, 32 * c),
                                )
                            mi += 1

                    for g in range(NGRP):
                        n, h = g // 2, g % 2
                        slot = (sg * NGRP + g) % RES_SLOTS
                        rs = res[:, slot * W_PAD : (slot + 1) * W_PAD]
                        nc.scalar.activation(
                            rs,
                            pts[g][:, :],
                            mybir.ActivationFunctionType.Identity,
                            bias=bt[:, :],
                            scale=st[:, :],
                        )
                        y0 = 128 * h + 32 * p + 4 * s
                        nc.scalar.dma_start(
                            out=o_d[n, :, y0 : y0 + 4, :].rearrange(
                                "o y w -> y o w"
                            ),
                            in_=rs[:, 0:256],
                        )
                    sg += 1
    nc.finalize()
    return nc


_NC_CACHE = {}


def _get_nc(mode):
    key = (mode, os.environ.get("BASS_CONV_REPS", "1"), os.environ.get("BASS_CONV_ORDER", "gc"))
    if key not in _NC_CACHE:
        _NC_CACHE[key] = build_nc(mode)
    return _NC_CACHE[key]


def _host_prep(weight, bias, mode):
    W32 = np.asarray(weight, dtype=np.float32)
    wmax = np.float32(np.max(np.abs(W32))) + np.float32(1e-12)
    k = np.round((W32 / wmax) * np.float32(QMAX))  # integral, in [-7, 7]

    if mode == "fp32":
        qW = (k / np.float32(QMAX)) * wmax  # bit-matches reference crossbar_map
        wvals = qW.astype(np.float32)
        scale = np.float32(1.0)
        np_dt = np.float32
    else:
        wvals = k  # exact small integers
        scale = np.float32(np.float64(wmax) / QMAX)
        np_dt = ml_dtypes.bfloat16

    # lhsT layout: [i, t*32 + o], t = 3*ky + kx
    lhsT = wvals.transpose(1, 2, 3, 0).reshape(C, 9 * C)  # [i,(ky,kx,o)]
    w_rep = np.tile(lhsT, (4, 1)).astype(np_dt)
    b_rep = np.tile(np.asarray(bias, dtype=np.float32)[:, None], (4, 1))
    s_rep = np.full((128, 1), scale, dtype=np.float32)
    return w_rep, b_rep, s_rep


def kernel(x, weight, bias):
    mode = _mode()
    x = np.asarray(x, dtype=np.float32)
    w_rep, b_rep, s_rep = _host_prep(weight, bias, mode)
    nc = _get_nc(mode)

    in_maps = []
    for cid in range(N_CORES):
        in_maps.append(
            {
                "x": np.ascontiguousarray(x[2 * cid : 2 * cid + 2]),
                "w": w_rep,
                "b": b_rep,
                "s": s_rep,
            }
        )
    r = run_bass_kernel_spmd(nc, in_maps, list(range(N_CORES)))
    out = np.empty((N_IMG, C, H, W), dtype=np.float32)
    for cid in range(N_CORES):
        out[2 * cid : 2 * cid + 2] = r.results[cid]["o"]
    return out
